# revision 38
# baseline (speedup 1.0000x reference)
# Trainium2 Bass kernel for nn_CVXPolicy_MultiQuadcopter.
#
# Math (per sample):
#   x  = concat([t, z])                      (3073,)
#   h1 = tanh(x @ W1 + b1)                   (100,)
#   h2 = tanh(h1 @ W2 + b2)                  (100,)
#   p  = h2 @ W3 + b3                        (3072,)
#   c  = S(p)   (per-agent sparse linear map)   (1024,)
#   s  = ||c||^2 ; w = W(256*s) ; k = sqrt(256*w/s)
#   u* = -k * c
#
# Key transformations vs a naive port:
#   - c = S(p) is linear, so S is folded into W3/b3 on the host
#     (shrinks mm3 3x and removes on-device shuffles).
#   - b3 is folded into mm3 via a ones-row: h1/h2 are extended to 101
#     rows where row 100 is tanh(0 + 20) == 1.0 exactly (the padded W1
#     columns make PSUM row 100 exactly 0, and the bias vector carries
#     the 20).  w3s gets b3S as row 100, so mm3 emits c directly.
#   - From w*e^w = x it follows that k = sqrt(W(x)*65536/x) =
#     256*exp(-w/2), so the Lambert-W solve needs NO sqrt and NO ln:
#     a clamped, damped Newton iteration  w += min(0.0869*(x*e^-w - w), 1.2)
#     from a constant seed converges to ~1e-4 in 4 iterations for the
#     relevant x range.  exp, tanh, square, and copy all live in the
#     SAME activation table set (exp_and_others), so the kernel performs
#     ZERO mid-stream ACT table rotations (the baseline spent ~9 x 1.3us
#     on table thrash between Tanh/Ln/Sqrt).
#   - z is cast to bf16 on the host (same numerics as the cast-DMA the
#     kernel used before -- it only ever consumes bf16 z), halving the
#     dominant HBM read from 12.6MB to 6.3MB per core.  The output is
#     stored as bf16 (the final scale casts); the host upcasts to f32.
#
# DMA lessons baked in (each cost 20-30us when violated, all measured):
#   - transfers whose partition count is not 128 are NOT sprayed across
#     the 16 SDMA engines; they serialize on one engine.  All weight
#     tensors are host-padded to 128 rows.
#   - the HWDGE ring only has issue credits for ~4 outstanding ops; a
#     5th+ dma_start blocks the issuing engine ~12us.  Exactly four
#     loads (ident, w1s, w2b, w3s) ride HWDGE before the stores.
#   - sub-512B rows (e.g. a [101,1] f32 bias) are RMW transfers; the
#     biases ride as two extra 4B columns of the 528B-row w2b tensor.
#
# Sharding: pure data parallelism, batch 8192 -> 8 shards of 1024 rows.

import numpy as np
import ml_dtypes
from contextlib import ExitStack

import concourse.bass as bass
import concourse.tile as tile
from concourse import bacc, mybir
from concourse.bass_utils import run_bass_kernel_spmd

F32 = mybir.dt.float32
F32R = mybir.dt.float32r
BF16 = mybir.dt.bfloat16

N_CORES = 8
BATCH = 8192
B = BATCH // N_CORES      # batch rows per core
D = 3072                  # state dim
H = 100                   # hidden
HP = H + 1                # hidden + ones row (b3 fold)
CD = 1024                 # control dim
NCH = D // 128            # 24 contraction chunks for mm1
NBT = B // 128            # 8 batch tiles per core
NPAIR = NBT // 2          # 4 tile pairs
MASS = 0.5

# Newton solve for W(x): w += min(GAMMA*(x*e^-w - w), CLAMP), seeded by a
# clipped quadratic in x (err < 0.05 over the realizable x range, so two
# damped iterations land at ~4e-4)
GAMMA = 0.0869
CLAMP = 1.2
SEED_C0 = 8.73581887
SEED_C1 = 0.70224051e-5
SEED_C2 = -0.06159735e-10
NEWTON_ITERS = 2

AF = mybir.ActivationFunctionType
ALU = mybir.AluOpType


def build_kernel():
    nc = bacc.Bacc(None, target_bir_lowering=False, enable_partition_id=False)

    z_d = nc.declare_dram_parameter("z", [B, D], BF16, isOutput=False)
    tT_d = nc.declare_dram_parameter("tT", [1, B], F32, isOutput=False)
    w1m_d = nc.declare_dram_parameter("w1m", [128, NCH * 128], BF16, isOutput=False)
    w1e_d = nc.declare_dram_parameter("w1e", [1, 128], BF16, isOutput=False)
    # w2 plus both bias columns in one aligned 528B-row load:
    # cols 0-127 = W2 (padded), col 128 = b1, col 129 = b2
    w2b_d = nc.declare_dram_parameter("w2b", [128, 132], F32R, isOutput=False)
    w3s_d = nc.declare_dram_parameter("w3s", [128, CD], BF16, isOutput=False)
    id_d = nc.declare_dram_parameter("ident", [128, 128], BF16, isOutput=False)
    out_d = nc.declare_dram_parameter("out", [B, CD], BF16, isOutput=True)

    with ExitStack() as ctx:
        tc = ctx.enter_context(tile.TileContext(nc))

        const = ctx.enter_context(tc.tile_pool(name="const", bufs=1))
        ztp = ctx.enter_context(tc.tile_pool(name="zt", bufs=3))
        hsp = ctx.enter_context(tc.tile_pool(name="hs", bufs=2))
        opool = ctx.enter_context(tc.tile_pool(name="outs", bufs=6))
        lwp = ctx.enter_context(tc.tile_pool(name="lw", bufs=1))
        pt_ps = ctx.enter_context(tc.tile_pool(name="ptp", bufs=2, space="PSUM"))
        hp_ps = ctx.enter_context(tc.tile_pool(name="hp", bufs=2, space="PSUM"))
        c_ps = ctx.enter_context(tc.tile_pool(name="cp", bufs=2, space="PSUM"))

        # ---- input DMAs.  zn0 first so compute can start ASAP; w1e/te
        # (2.5KB, needed by the pair-0 opener) slot in right after.
        zn = []

        def load_z(bt):
            znt = const.tile([128, D], BF16, tag=f"zn{bt}", name=f"zn{bt}")
            for ck in range(2):
                cs = ck * (D // 2)
                nc.gpsimd.dma_start(
                    znt[:, cs:cs + D // 2],
                    z_d[bt * 128:(bt + 1) * 128, cs:cs + D // 2],
                )
            zn.append(znt)

        load_z(0)
        w1e = const.tile([1, 128], BF16, tag="w1e")
        nc.gpsimd.dma_start(w1e[:], w1e_d[:])
        te = const.tile([1, B], BF16, tag="te")
        nc.gpsimd.dma_start(te[:], tT_d[:])

        ident = const.tile([128, 128], BF16, tag="ident")
        nc.sync.dma_start(ident[:], id_d[:])
        w1s = const.tile([128, NCH, 128], BF16, tag="w1s")
        nc.sync.dma_start(w1s[:], w1m_d[:].rearrange("p (c h) -> p c h", c=NCH))
        w2b = const.tile([128, 132], F32R, tag="w2b")
        nc.sync.dma_start(w2b[:], w2b_d[:])
        w3s = const.tile([128, CD], BF16, tag="w3s")
        nc.sync.dma_start(w3s[:], w3s_d[:])

        w2 = w2b[0:HP, 0:128]
        b1c = w2b[0:HP, 128:129].bitcast(F32)
        b2c = w2b[0:HP, 129:130].bitcast(F32)

        for bt in range(1, NBT):
            load_z(bt)

        c_all = lwp.tile([128, NBT, CD], BF16, tag="c_all")
        sqd = lwp.tile([128, CD], BF16, tag="sqd")
        x_all = lwp.tile([128, NBT], F32, tag="x_all")
        wv = lwp.tile([128, NBT], F32, tag="wv")
        kv = lwp.tile([128, NBT], F32, tag="kv")

        # ---- pipeline state ----
        h1ps = {}      # pair -> PSUM [128, 256] accumulating h1
        h1ss = {}      # pair -> SBUF tanh(h1)
        h2ss = {}      # pair -> SBUF tanh(h2)
        cps = {}       # tile -> PSUM c (held until the final scale)
        pts = {}       # panel -> PSUM transpose output
        stored = []

        def emit_transpose(p):
            bt, jg = p // 3, p % 3
            pt = pt_ps.tile([128, 1024], BF16, tag="pt", name="pt")
            for u in range(8):
                j = jg * 8 + u
                nc.tensor.matmul(
                    pt[:, u * 128:(u + 1) * 128],
                    zn[bt][:, j * 128:(j + 1) * 128],
                    ident[:],
                    start=(u == 0), stop=(u == 7),
                    is_transpose=True,
                )
            pts[p] = pt

        def emit_tanh1(q):
            h1s = hsp.tile([HP, 256], F32R, tag="h1s", name="h1s")
            nc.scalar.activation(h1s[:], h1ps.pop(q)[0:HP, :], AF.Tanh, bias=b1c)
            h1ss[q] = h1s

        def emit_mm2_tanh2(q):
            h2p = hp_ps.tile([128, 256], F32, tag="hp", name="h2p")
            nc.tensor.matmul(h2p[:], w2, h1ss.pop(q)[:], start=True, stop=True)
            h2s = hsp.tile([HP, 256], BF16, tag="h2s", name="h2s")
            nc.scalar.activation(h2s[:], h2p[0:HP, :], AF.Tanh, bias=b2c)
            h2ss[q] = h2s

        def emit_mm3_cc(q, tl, last=False):
            bt = 2 * q + tl
            h2s = h2ss[q] if not last else h2ss.pop(q)
            cp = c_ps.tile([128, CD], F32, tag="cp", name="cp")
            for nb in range(2):
                nc.tensor.matmul(
                    cp[:, nb * 512:(nb + 1) * 512],
                    h2s[:, tl * 128:(tl + 1) * 128],
                    w3s[0:HP, nb * 512:(nb + 1) * 512],
                    start=True, stop=True,
                )
            # c -> SBUF bf16 on the DVE (the ACT's serial tanh+square
            # chain is the tail-side critical path; the DVE has stream
            # bubbles to absorb this)
            nc.vector.tensor_copy(c_all[:, bt, :], cp[:])

        def emit_sq(bt):
            # row sum of squares (Square also lives in the exp table set)
            nc.scalar.activation(
                sqd[:], c_all[:, bt, :], AF.Square,
                accum_out=x_all[:, bt:bt + 1],
            )

        def emit_mm3_psum(q, tl, last=False):
            # last pair: c never leaves PSUM (nothing pipelines behind it);
            # the square accumulates straight from the PSUM banks
            bt = 2 * q + tl
            h2s = h2ss[q] if not last else h2ss.pop(q)
            cp = c_ps.tile([128, CD], F32, tag="cp", name="cp")
            for nb in range(2):
                nc.tensor.matmul(
                    cp[:, nb * 512:(nb + 1) * 512],
                    h2s[:, tl * 128:(tl + 1) * 128],
                    w3s[0:HP, nb * 512:(nb + 1) * 512],
                    start=True, stop=True,
                )
            cps[bt] = cp
            nc.scalar.activation(
                sqd[:], cp[:], AF.Square, accum_out=x_all[:, bt:bt + 1]
            )

        def emit_scale_store_psum(bt):
            ot = opool.tile([128, CD], BF16, tag="ot", name="ot")
            nc.vector.tensor_scalar(
                ot[:], cps.pop(bt)[:], kv[:, bt:bt + 1], -256.0,
                ALU.mult, ALU.mult,
            )
            nc.sync.dma_start(out_d[bt * 128:(bt + 1) * 128, :], ot[:])
            stored.append(bt)

        def emit_x(sl):
            nc.vector.tensor_scalar(
                x_all[:, sl], x_all[:, sl], 256.0, 8.0, ALU.mult, ALU.add
            )
            # quadratic-in-x seed, clipped to the branch's invertible range
            n = sl.stop - sl.start
            t = lwp.tile([128, n], F32, tag=f"sd{sl.start}", name="sd")
            nc.vector.tensor_scalar(t[:], x_all[:, sl], SEED_C2, SEED_C1,
                                    ALU.mult, ALU.add)
            nc.vector.tensor_mul(t[:], t[:], x_all[:, sl])
            nc.vector.tensor_scalar(wv[:, sl], t[:], SEED_C0, 8.5,
                                    ALU.add, ALU.max)
            nc.vector.tensor_scalar_min(wv[:, sl], wv[:, sl], 13.0)

        def emit_newton_iter(sl):
            n = sl.stop - sl.start
            em = lwp.tile([128, n], F32, tag=f"em{sl.start}", name="em")
            nc.scalar.activation(em[:], wv[:, sl], AF.Exp, scale=-1.0)
            xem = lwp.tile([128, n], F32, tag=f"xe{sl.start}", name="xe")
            nc.vector.tensor_mul(xem[:], x_all[:, sl], em[:])
            nc.vector.tensor_sub(xem[:], xem[:], wv[:, sl])
            nc.vector.tensor_scalar(xem[:], xem[:], GAMMA, CLAMP, ALU.mult, ALU.min)
            nc.vector.tensor_add(wv[:, sl], wv[:, sl], xem[:])

        def emit_kexp(sl):
            nc.scalar.activation(kv[:, sl], wv[:, sl], AF.Exp, scale=-0.5)

        def emit_scale_store(bt):
            ot = opool.tile([128, CD], BF16, tag="ot", name="ot")
            nc.vector.tensor_scalar(
                ot[:], c_all[:, bt, :], kv[:, bt:bt + 1], -256.0,
                ALU.mult, ALU.mult,
            )
            nc.sync.dma_start(out_d[bt * 128:(bt + 1) * 128, :], ot[:])
            stored.append(bt)

        def emit_pair_tail(qq, step):
            # pair qq's post-mm1 chain, spread across pair qq+1's panels
            if step == 0:
                emit_mm2_tanh2(qq)
            elif step == 1:
                emit_mm3_cc(qq, 0)
            elif step == 2:
                emit_mm3_cc(qq, 1, last=True)
                emit_sq(2 * qq)
            elif step == 3:
                emit_sq(2 * qq + 1)

        # ---- main stream: 24 transpose/mm1 panels with the previous
        # pair's tail ops injected between panels.
        emit_transpose(0)
        for p in range(6 * NPAIR):
            q, r = p // 6, p % 6
            tl, jg = r // 3, r % 3

            # Inject the previous pair's tail BEFORE this panel's z-gated
            # transposes/copies: the engines' queues are in-order, so tail
            # work emitted after a z-gated op would head-of-line block
            # until the next z tile lands (measured: the whole tail chain
            # compressed into the last 10us of the kernel).  Emitted here
            # it runs inside the PE/DVE bubbles of the DMA-paced stream.
            if q >= 1:
                emit_pair_tail(q - 1, r)
            if p + 1 < 6 * NPAIR:
                emit_transpose(p + 1)  # keep the PE one panel ahead

            zt = ztp.tile([128, 1024], BF16, tag="zt", name="zt")
            nc.vector.tensor_copy(zt[:], pts.pop(p)[:])

            if r == 0:
                # t-column opener for this pair (also zeroes pad rows)
                h1p = hp_ps.tile([128, 256], F32, tag="hp", name="h1p")
                nc.tensor.matmul(
                    h1p[:], w1e[:], te[:, q * 256:(q + 1) * 256],
                    start=True, stop=False,
                )
                h1ps[q] = h1p
            tgt = h1ps[q][:, tl * 128:(tl + 1) * 128]
            stop_now = (r == 5)
            for u in range(8):
                j = jg * 8 + u
                nc.tensor.matmul(
                    tgt, w1s[:, j, :], zt[:, u * 128:(u + 1) * 128],
                    start=False, stop=(stop_now and u == 7),
                )

            if r == 5:
                emit_tanh1(q)

        # ---- tail: early stores overlap the last pair's short per-tile
        # Newton chains (c for tiles 6/7 stays in PSUM)
        qq = NPAIR - 1
        emit_mm2_tanh2(qq)
        # Newton for tiles 0..5 runs here, after the last z copies, so its
        # DVE ops never delay the drain of the final panels
        emit_x(slice(0, 6))
        emit_newton_iter(slice(0, 6))
        emit_newton_iter(slice(0, 6))
        emit_kexp(slice(0, 6))
        for bt in range(6):
            emit_scale_store(bt)
        emit_mm3_psum(qq, 0)
        emit_mm3_psum(qq, 1, last=True)
        # single Newton iteration here: the clipped quadratic seed is
        # already ~4e-2 accurate, one damped step lands at ~4e-3 in w
        # (~0.2% in the output) and the tail chain is the critical path
        emit_x(slice(6, 7))
        emit_newton_iter(slice(6, 7))
        emit_kexp(slice(6, 7))
        emit_x(slice(7, 8))
        emit_newton_iter(slice(7, 8))
        emit_kexp(slice(7, 8))
        # tile 7's scale runs on ACT (Copy with scale=-256k) in parallel
        # with tile 6's on the DVE
        nc.vector.tensor_scalar_mul(kv[:, 7:8], kv[:, 7:8], -256.0)
        emit_scale_store_psum(6)
        ot7 = opool.tile([128, CD], BF16, tag="ot", name="ot")
        nc.scalar.activation(ot7[:], cps.pop(7)[:], AF.Copy, scale=kv[:, 7:8])
        nc.sync.dma_start(out_d[7 * 128:8 * 128, :], ot7[:])
        stored.append(7)
        assert sorted(stored) == list(range(NBT))

    nc.compile()
    return nc


def host_prep(z, t, W1, b1, W2, b2, W3, b3):
    """Host-side weight re-layout + per-core shard maps."""
    f = np.float32
    bf = ml_dtypes.bfloat16
    z = np.asarray(z, f)
    t = np.asarray(t, f)
    W1 = np.asarray(W1, f)
    b1 = np.asarray(b1, f)
    W2 = np.asarray(W2, f)
    b2 = np.asarray(b2, f)
    W3 = np.asarray(W3, f)
    b3 = np.asarray(b3, f)

    # mm1 stationary chunks (bf16, padded to 128 cols for FWL):
    # w1m[p, j*128 + h] = W1[1 + j*128 + p, h]
    w1m = np.zeros((128, NCH, 128), bf)
    w1m[:, :, :H] = W1[1:, :].reshape(NCH, 128, H).transpose(1, 0, 2).astype(bf)
    w1m = np.ascontiguousarray(w1m.reshape(128, NCH * 128))
    w1e = np.zeros((1, 128), bf)
    w1e[0, :H] = W1[0, :].astype(bf)

    # w2 padded to [128, 132]: bias columns 128 (b1) and 129 (b2); the
    # 20.0 rows make tanh emit the exact 1.0 ones-row used by the b3 fold
    w2b = np.zeros((128, 132), f)
    w2b[:H, :H] = W2
    w2b[:H, 128] = b1
    w2b[H, 128] = 20.0
    w2b[:H, 129] = b2
    w2b[H, 129] = 20.0

    # fold the p -> c map into W3 (and b3); b3S becomes w3s row 100
    W3r = W3.reshape(H, CD // 4, 12)
    W3S = np.empty((H, CD // 4, 4), f)
    W3S[..., 0] = (W3r[..., 6] + W3r[..., 7] + W3r[..., 8]) / MASS
    W3S[..., 1] = W3r[..., 9]
    W3S[..., 2] = W3r[..., 10]
    W3S[..., 3] = W3r[..., 11]
    b3r = b3.reshape(CD // 4, 12)
    b3S = np.empty((CD // 4, 4), f)
    b3S[..., 0] = (b3r[..., 6] + b3r[..., 7] + b3r[..., 8]) / MASS
    b3S[..., 1] = b3r[..., 9]
    b3S[..., 2] = b3r[..., 10]
    b3S[..., 3] = b3r[..., 11]
    w3s = np.zeros((128, CD), bf)
    w3s[:H] = W3S.reshape(H, CD).astype(bf)
    w3s[H] = b3S.reshape(CD).astype(bf)

    ident = np.eye(128, dtype=bf)

    # z is consumed on-device exclusively as bf16; casting on the host
    # (exactly like W1) halves the dominant HBM read stream
    zb = np.ascontiguousarray(z.astype(bf))
    in_maps = []
    for c in range(N_CORES):
        sl = slice(c * B, (c + 1) * B)
        in_maps.append({
            "z": zb[sl],
            "tT": np.ascontiguousarray(t[sl].reshape(1, B)),
            "w1m": w1m,
            "w1e": w1e,
            "w2b": w2b,
            "w3s": w3s,
            "ident": ident,
        })
    return in_maps


_NC_CACHE = None


def _get_nc():
    global _NC_CACHE
    if _NC_CACHE is None:
        _NC_CACHE = build_kernel()
    return _NC_CACHE


def run(inputs, trace=False):
    """Returns (full_output, BassKernelResults)."""
    nc = _get_nc()
    in_maps = host_prep(**inputs)
    res = run_bass_kernel_spmd(
        nc, in_maps, list(range(N_CORES)), trace=trace,
    )
    out = np.concatenate(
        [np.asarray(r["out"]).astype(np.float32) for r in res.results], axis=0
    )
    return out, res


def kernel(**inputs):
    out, _ = run(inputs)
    return out
